# revision 9
# baseline (speedup 1.0000x reference)
"""HebbianMLP forward + eligibility/Hebbian trace update on 8 TRN2 NeuronCores.

Data-parallel over batch (B=4096 -> 512 rows/core), weights replicated.
Forward GEMMs and trace GEMMs run in float32r (FP22 on the PE, full rate).
Trace partial products are summed across cores with f32 ReduceScatter
(row-sharded), so each core produces a 256-row shard of the e/heb updates.
The dopa / W_new scalars come from a tiny AllReduce of per-core tanh sums.

Hardware rule honored throughout: every fp32r matmul operand must be
DMA-written (the BIR verifier rejects engine-written fp32r matmul inputs),
so activations bounce through DRAM and the inter-layer transposes are a
dedicated DMA -> PE-transpose -> relu -> DMA phase.
"""

import sys

if "/opt/trn_rl_repo" not in sys.path:
    sys.path.insert(0, "/opt/trn_rl_repo")

from contextlib import ExitStack

import numpy as np

from concourse import bacc, bass, tile
from concourse import mybir
from concourse import bass_utils
from concourse.bass import ds
from concourse.kernels.tile_matmul import matmul_tile_kernel

B = 4096
D_IN = 2048
H0 = 2048
H1 = 2048
D_OUT = 2050
D_OUT_PAD = 2560           # 5 x 512 so every tile is full
ALPHA = 0.3
CLAMP = 0.3
N_CORES = 8
BL = B // N_CORES          # 512 local batch rows
RS_ROWS = D_IN // N_CORES  # 256 rows of each trace matrix per core

f32 = mybir.dt.float32
f32r = mybir.dt.float32r

_CACHE = {}


def _store_relu_post(h_dram):
    """post_mxn_tile_fn: write relu(sbuf product tile) to h_dram (batch-major)."""

    def post(nc, sbuf, md, _):
        pool = _CACHE["relu_pool"]
        ns = md.n_slice_size
        t = pool.tile(list(sbuf.shape), f32r, tag=f"relu_{sbuf.shape[-1]}")
        nc.scalar.activation(
            t[:, :, :ns], sbuf, mybir.ActivationFunctionType.Relu
        )
        m0 = md.m_tile_idx * md.m_tile
        n0 = md.n_tile_idx * md.n_tile
        dst = h_dram[ds(m0, md.m_tile), ds(n0, ns)].rearrange(
            "(s p) n -> p s n", p=128
        )
        nc.sync.dma_start(dst, t[:, :, :ns])

    return post


def _transpose_relu(nc, tc, ctx, src, dst, ident, rows, cols, name):
    """dst[j, b] = relu(src[b, j].T) via PE transpose, all tiles DMA-backed."""
    pool = ctx.enter_context(tc.tile_pool(name=f"tp_{name}", bufs=6))
    tpsum = ctx.enter_context(
        tc.tile_pool(name=f"tpp_{name}", bufs=4, space="PSUM")
    )
    for jt in range(cols // 128):
        for bt in range(rows // 128):
            tin = pool.tile([128, 128], f32r, tag="tin")
            nc.sync.dma_start(tin[:], src[ds(bt * 128, 128), ds(jt * 128, 128)])
            pt = tpsum.tile([128, 128], f32r, tag="pt")
            nc.tensor.transpose(pt[:], tin[:], ident[:])
            tt = pool.tile([128, 128], f32r, tag="tt")
            nc.scalar.activation(
                tt[:], pt[:], mybir.ActivationFunctionType.Relu
            )
            nc.sync.dma_start(dst[ds(jt * 128, 128), ds(bt * 128, 128)], tt[:])


def build():
    if "nc" in _CACHE:
        return _CACHE["nc"]

    nc = bacc.Bacc(
        "TRN2",
        target_bir_lowering=False,
        debug=False,
        num_devices=N_CORES,
    )

    # ---- I/O -----------------------------------------------------------
    xT = nc.dram_tensor("xT", [D_IN, BL], f32r, kind="ExternalInput")
    x = nc.dram_tensor("x", [BL, D_IN], f32r, kind="ExternalInput")
    w0 = nc.dram_tensor("w0", [D_IN, H0], f32r, kind="ExternalInput")
    w1 = nc.dram_tensor("w1", [H0, H1], f32r, kind="ExternalInput")
    w2 = nc.dram_tensor("w2", [H1, D_OUT_PAD], f32r, kind="ExternalInput")
    ident = nc.dram_tensor("ident", [128, 128], f32r, kind="ExternalInput")
    e0s = nc.dram_tensor("e0s", [RS_ROWS, H0], f32, kind="ExternalInput")
    e1s = nc.dram_tensor("e1s", [RS_ROWS, H1], f32, kind="ExternalInput")
    e2s = nc.dram_tensor("e2s", [RS_ROWS, D_OUT_PAD], f32, kind="ExternalInput")
    hb0s = nc.dram_tensor("hb0s", [RS_ROWS, H0], f32, kind="ExternalInput")
    hb1s = nc.dram_tensor("hb1s", [RS_ROWS, H1], f32, kind="ExternalInput")
    hb2s = nc.dram_tensor("hb2s", [RS_ROWS, D_OUT_PAD], f32, kind="ExternalInput")

    y_out = nc.dram_tensor("y_out", [BL, 2048], f32r, kind="ExternalOutput")
    en0 = nc.dram_tensor("en0", [RS_ROWS, H0], f32, kind="ExternalOutput")
    en1 = nc.dram_tensor("en1", [RS_ROWS, H1], f32, kind="ExternalOutput")
    en2 = nc.dram_tensor("en2", [RS_ROWS, D_OUT_PAD], f32, kind="ExternalOutput")
    hbn0 = nc.dram_tensor("hbn0", [RS_ROWS, H0], f32, kind="ExternalOutput")
    hbn1 = nc.dram_tensor("hbn1", [RS_ROWS, H1], f32, kind="ExternalOutput")
    hbn2 = nc.dram_tensor("hbn2", [RS_ROWS, D_OUT_PAD], f32, kind="ExternalOutput")
    wn_out = nc.dram_tensor("wn_out", [1, 1], f32, kind="ExternalOutput")
    dopa_out = nc.dram_tensor("dopa_out", [1, 1], f32, kind="ExternalOutput")

    groups = [list(range(N_CORES))]

    with tile.TileContext(nc) as tc:
        with ExitStack() as octx:
            dram = octx.enter_context(tc.tile_pool(name="dram", bufs=1, space="DRAM"))
            relu_pool = octx.enter_context(tc.tile_pool(name="relu_pool", bufs=3))
            _CACHE["relu_pool"] = relu_pool

            h0x = dram.tile([BL, H0], f32r)    # pre-activation, batch-major
            h0b = dram.tile([BL, H0], f32r)    # relu, batch-major
            h0T = dram.tile([H0, BL], f32r)    # relu, feature-major
            h1x = dram.tile([BL, H1], f32r)
            h1b = dram.tile([BL, H1], f32r)
            h1T = dram.tile([H1, BL], f32r)
            y1 = dram.tile([BL, D_OUT_PAD], f32r)
            p0 = dram.tile([D_IN, H0], f32)
            p1 = dram.tile([H0, H1], f32)
            p2 = dram.tile([H1, D_OUT_PAD], f32)
            rs0 = dram.tile([RS_ROWS, H0], f32)
            rs1 = dram.tile([RS_ROWS, H1], f32)
            rs2 = dram.tile([RS_ROWS, D_OUT_PAD], f32)
            ar_in = dram.tile([1, 2], f32)
            ar_out = dram.tile([1, 2], f32, addr_space="Shared")

            idsb_pool = octx.enter_context(tc.tile_pool(name="idsb_pool", bufs=1))
            idsb = idsb_pool.tile([128, 128], f32r)
            nc.sync.dma_start(idsb[:], ident.ap())

            # ---- L1: h0x = x @ w0, plus relu copy h0b
            matmul_tile_kernel(
                tc, xT.ap(), w0.ap(), h0x,
                post_mxn_tile_fn=_store_relu_post(h0b),
            )

            # ---- transpose: h0T = relu(h0x).T
            with ExitStack() as ctx:
                _transpose_relu(nc, tc, ctx, h0x, h0T, idsb, BL, H0, "t0")

            # ---- trace 0 partial: p0 = x.T @ h0x
            matmul_tile_kernel(tc, x.ap(), h0x, p0)
            nc.gpsimd.collective_compute(
                "ReduceScatter",
                mybir.AluOpType.add,
                replica_groups=groups,
                ins=[p0.opt()],
                outs=[rs0.opt()],
            )

            # ---- L2: h1x = relu(h0x) @ w1 = h0T.T @ w1
            matmul_tile_kernel(
                tc, h0T, w1.ap(), h1x,
                post_mxn_tile_fn=_store_relu_post(h1b),
            )

            # ---- transpose: h1T = relu(h1x).T
            with ExitStack() as ctx:
                _transpose_relu(nc, tc, ctx, h1x, h1T, idsb, BL, H1, "t1")

            # ---- trace 1 partial: p1 = relu(h0x).T @ h1x
            matmul_tile_kernel(tc, h0b, h1x, p1)
            nc.gpsimd.collective_compute(
                "ReduceScatter",
                mybir.AluOpType.add,
                replica_groups=groups,
                ins=[p1.opt()],
                outs=[rs1.opt()],
            )

            # ---- L3: y1 = relu(h1x) @ w2
            matmul_tile_kernel(tc, h1T, w2.ap(), y1)

            # ---- trace 2 partial: p2 = relu(h1x).T @ y1
            matmul_tile_kernel(tc, h1b, y1, p2)
            nc.gpsimd.collective_compute(
                "ReduceScatter",
                mybir.AluOpType.add,
                replica_groups=groups,
                ins=[p2.opt()],
                outs=[rs2.opt()],
            )

            # ---- y output: first 2048 columns
            nc.sync.dma_start(y_out.ap(), y1[:, 0:2048])

            # ---- local tanh sums for dopa (y1 col 2047) / W_new (col 2048)
            with ExitStack() as ctx:
                sc_pool = ctx.enter_context(tc.tile_pool(name="sc_pool", bufs=1))
                sc_psum = ctx.enter_context(
                    tc.tile_pool(name="sc_psum", bufs=1, space="PSUM")
                )
                ycols = sc_pool.tile([128, BL // 128, 2], f32r)
                nc.sync.dma_start(
                    ycols[:],
                    y1[:, 2047:2049].rearrange("(a p) c -> p a c", p=128),
                )
                # f32 (not f32r) tiles: this matmul runs in plain fp32 so the
                # engine-written tanh output is a legal operand
                tcols = sc_pool.tile([128, BL // 128, 2], f32)
                nc.scalar.activation(
                    tcols[:], ycols[:], mybir.ActivationFunctionType.Tanh
                )
                ones = sc_pool.tile([128, 1], f32)
                nc.any.memset(ones[:], 1.0)
                psc = sc_psum.tile([1, 2], f32)
                for a in range(BL // 128):
                    nc.tensor.matmul(
                        psc[:],
                        ones[:],
                        tcols[:, a, :],
                        start=(a == 0),
                        stop=(a == BL // 128 - 1),
                    )
                ssum = sc_pool.tile([1, 2], f32)
                nc.any.tensor_copy(ssum[:], psc[:])
                nc.sync.dma_start(ar_in[:], ssum[:])

            nc.gpsimd.collective_compute(
                "AllReduce",
                mybir.AluOpType.add,
                replica_groups=groups,
                ins=[ar_in.opt()],
                outs=[ar_out.opt()],
            )

            # ---- scalars: divide by B, write outputs, broadcast dopa
            with ExitStack() as ctx:
                sc2_pool = ctx.enter_context(tc.tile_pool(name="sc2_pool", bufs=1))
                gsum = sc2_pool.tile([1, 2], f32)
                nc.sync.dma_start(gsum[:], ar_out[:])
                gmean = sc2_pool.tile([1, 2], f32)
                nc.scalar.mul(gmean[:], gsum[:], 1.0 / B)
                nc.sync.dma_start(dopa_out.ap(), gmean[:, 0:1])
                nc.sync.dma_start(wn_out.ap(), gmean[:, 1:2])
                # round-trip through DRAM to broadcast dopa across partitions
                dopa_dram = dram.tile([1, 1], f32)
                nc.sync.dma_start(dopa_dram[:], gmean[:, 0:1])
                dbr = sc2_pool.tile([128, 1], f32)
                nc.sync.dma_start(dbr[:], dopa_dram[:].partition_broadcast(128))

                # ---- epilogue: e_n = (1-a)*e + a*P ; heb_n = clip(heb + dopa*e_n)
                ep_pool = ctx.enter_context(tc.tile_pool(name="ep_pool", bufs=3))
                for rs_t, e_t, hb_t, en_t, hbn_t, C in (
                    (rs0, e0s, hb0s, en0, hbn0, H0),
                    (rs1, e1s, hb1s, en1, hbn1, H1),
                    (rs2, e2s, hb2s, en2, hbn2, D_OUT_PAD),
                ):
                    for r in range(RS_ROWS // 128):
                        rsl = slice(r * 128, (r + 1) * 128)
                        pt = ep_pool.tile([128, C], f32, tag="pt")
                        et = ep_pool.tile([128, C], f32, tag="et")
                        hbt = ep_pool.tile([128, C], f32, tag="hbt")
                        nc.sync.dma_start(pt[:], rs_t[rsl, :])
                        nc.sync.dma_start(et[:], e_t.ap()[rsl, :])
                        nc.sync.dma_start(hbt[:], hb_t.ap()[rsl, :])
                        ent = ep_pool.tile([128, C], f32, tag="ent")
                        nc.vector.tensor_scalar_mul(ent[:], pt[:], ALPHA)
                        nc.vector.tensor_scalar_mul(et[:], et[:], 1.0 - ALPHA)
                        nc.vector.tensor_tensor(
                            ent[:], ent[:], et[:], mybir.AluOpType.add
                        )
                        nc.sync.dma_start(en_t.ap()[rsl, :], ent[:])
                        hdt = ep_pool.tile([128, C], f32, tag="hdt")
                        nc.vector.tensor_scalar_mul(hdt[:], ent[:], dbr[:, 0:1])
                        nc.vector.tensor_tensor(
                            hdt[:], hdt[:], hbt[:], mybir.AluOpType.add
                        )
                        nc.vector.tensor_scalar(
                            hdt[:],
                            hdt[:],
                            CLAMP,
                            -CLAMP,
                            op0=mybir.AluOpType.min,
                            op1=mybir.AluOpType.max,
                        )
                        nc.sync.dma_start(hbn_t.ap()[rsl, :], hdt[:])

    nc.compile()
    _CACHE["nc"] = nc
    return nc


def _prep_in_maps(x, x2h0, h02h1, h12y, e_x2h0, e_h02h1, e_h12y,
                  heb_x2h0, heb_h02h1, heb_h12y, W):
    W_ = float(np.asarray(W))
    f = np.float32

    def eff(w, heb):
        w = np.asarray(w, dtype=f)
        if W_ != 0.0:
            w = (w + f(W_) * np.asarray(heb, dtype=f)).astype(f)
        return np.ascontiguousarray(w)

    def padcols(a, width):
        a = np.asarray(a, dtype=f)
        out = np.zeros((a.shape[0], width), dtype=f)
        out[:, : a.shape[1]] = a
        return out

    w0 = eff(x2h0, heb_x2h0)
    w1 = eff(h02h1, heb_h02h1)
    w2 = padcols(eff(h12y, heb_h12y), D_OUT_PAD)

    x = np.asarray(x, dtype=f)
    e0 = np.asarray(e_x2h0, dtype=f)
    e1 = np.asarray(e_h02h1, dtype=f)
    e2 = padcols(e_h12y, D_OUT_PAD)
    hb0 = np.asarray(heb_x2h0, dtype=f)
    hb1 = np.asarray(heb_h02h1, dtype=f)
    hb2 = padcols(heb_h12y, D_OUT_PAD)
    ident = np.eye(128, dtype=f)

    in_maps = []
    for c in range(N_CORES):
        bsl = slice(c * BL, (c + 1) * BL)
        rsl = slice(c * RS_ROWS, (c + 1) * RS_ROWS)
        xc = np.ascontiguousarray(x[bsl])
        in_maps.append({
            "xT": np.ascontiguousarray(xc.T),
            "x": xc,
            "w0": w0,
            "w1": w1,
            "w2": w2,
            "ident": ident,
            "e0s": np.ascontiguousarray(e0[rsl]),
            "e1s": np.ascontiguousarray(e1[rsl]),
            "e2s": np.ascontiguousarray(e2[rsl]),
            "hb0s": np.ascontiguousarray(hb0[rsl]),
            "hb1s": np.ascontiguousarray(hb1[rsl]),
            "hb2s": np.ascontiguousarray(hb2[rsl]),
        })
    return in_maps


def _assemble(results):
    cat = lambda key: np.concatenate(
        [results[c][key] for c in range(N_CORES)], axis=0
    )
    y = cat("y_out")
    e0 = cat("en0")
    e1 = cat("en1")
    e2 = cat("en2")[:, :D_OUT]
    h0 = cat("hbn0")
    h1 = cat("hbn1")
    h2 = cat("hbn2")[:, :D_OUT]
    w_new = np.float32(results[0]["wn_out"][0, 0])
    dopa = np.float32(results[0]["dopa_out"][0, 0])
    return (y, w_new, dopa,
            np.ascontiguousarray(e0), np.ascontiguousarray(e1),
            np.ascontiguousarray(e2), np.ascontiguousarray(h0),
            np.ascontiguousarray(h1), np.ascontiguousarray(h2))


def run(inputs, trace=False, **kw):
    nc = build()
    in_maps = _prep_in_maps(**inputs)
    res = bass_utils.run_bass_kernel_spmd(
        nc, in_maps, core_ids=list(range(N_CORES)), trace=trace, **kw
    )
    return _assemble(res.results), res


def kernel(**inputs):
    out, _ = run(inputs)
    return out


# revision 18
# speedup vs baseline: 1.0118x; 1.0118x over previous
"""HebbianMLP forward + eligibility/Hebbian trace update on 8 TRN2 NeuronCores.

Data-parallel over batch (B=4096 -> 512 rows/core), weights replicated.
Forward GEMMs and trace GEMMs run in float32r (FP22 on the PE, full rate).
Trace partial products are summed across cores with f32 ReduceScatter
(row-sharded), so each core produces a 256-row shard of the e/heb updates.
The dopa / W_new scalars come from a tiny AllReduce of per-core tanh sums.

Hardware rule honored throughout: every fp32r matmul operand must be
DMA-written (the BIR verifier rejects engine-written fp32r matmul inputs),
so activations bounce through DRAM and the inter-layer transposes are a
dedicated DMA -> PE-transpose -> relu -> DMA phase.
"""

import sys

if "/opt/trn_rl_repo" not in sys.path:
    sys.path.insert(0, "/opt/trn_rl_repo")

from contextlib import ExitStack

import numpy as np

from concourse import bacc, bass, tile
from concourse import mybir
from concourse import bass_utils
from concourse.bass import ds
from concourse.kernels.tile_matmul import matmul_tile_kernel

B = 4096
D_IN = 2048
H0 = 2048
H1 = 2048
D_OUT = 2050
D_OUT_PAD = 2560           # 5 x 512 so every tile is full
ALPHA = 0.3
CLAMP = 0.3
N_CORES = 8
BL = B // N_CORES          # 512 local batch rows
RS_ROWS = D_IN // N_CORES  # 256 rows of each trace matrix per core

f32 = mybir.dt.float32
f32r = mybir.dt.float32r

_CACHE = {}


def _store_relu_post(h_dram):
    """post_mxn_tile_fn: write relu(sbuf product tile) to h_dram (batch-major)."""

    def post(nc, sbuf, md, _):
        pool = _CACHE["relu_pool"]
        ns = md.n_slice_size
        t = pool.tile(list(sbuf.shape), f32r, tag=f"relu_{sbuf.shape[-1]}")
        nc.scalar.activation(
            t[:, :, :ns], sbuf, mybir.ActivationFunctionType.Relu
        )
        m0 = md.m_tile_idx * md.m_tile
        n0 = md.n_tile_idx * md.n_tile
        dst = h_dram[ds(m0, md.m_tile), ds(n0, ns)].rearrange(
            "(s p) n -> p s n", p=128
        )
        nc.sync.dma_start(dst, t[:, :, :ns])

    return post


def _transpose_relu(nc, tc, ctx, src, dst, ident, rows, cols, name):
    """dst[j, b] = relu(src[b, j].T) via PE transpose, all tiles DMA-backed."""
    pool = ctx.enter_context(tc.tile_pool(name=f"tp_{name}", bufs=6))
    tpsum = ctx.enter_context(
        tc.tile_pool(name=f"tpp_{name}", bufs=4, space="PSUM")
    )
    for jt in range(cols // 128):
        for bt in range(rows // 128):
            tin = pool.tile([128, 128], f32r, tag="tin")
            nc.sync.dma_start(tin[:], src[ds(bt * 128, 128), ds(jt * 128, 128)])
            pt = tpsum.tile([128, 128], f32r, tag="pt")
            nc.tensor.transpose(pt[:], tin[:], ident[:])
            tt = pool.tile([128, 128], f32r, tag="tt")
            nc.scalar.activation(
                tt[:], pt[:], mybir.ActivationFunctionType.Relu
            )
            nc.sync.dma_start(dst[ds(jt * 128, 128), ds(bt * 128, 128)], tt[:])


def build():
    if "nc" in _CACHE:
        return _CACHE["nc"]

    nc = bacc.Bacc(
        "TRN2",
        target_bir_lowering=False,
        debug=False,
        num_devices=N_CORES,
    )

    # ---- I/O -----------------------------------------------------------
    xT = nc.dram_tensor("xT", [D_IN, BL], f32r, kind="ExternalInput")
    x = nc.dram_tensor("x", [BL, D_IN], f32r, kind="ExternalInput")
    w0 = nc.dram_tensor("w0", [D_IN, H0], f32r, kind="ExternalInput")
    w1 = nc.dram_tensor("w1", [H0, H1], f32r, kind="ExternalInput")
    w2 = nc.dram_tensor("w2", [H1, D_OUT_PAD], f32r, kind="ExternalInput")
    ident = nc.dram_tensor("ident", [128, 128], f32r, kind="ExternalInput")
    e0s = nc.dram_tensor("e0s", [RS_ROWS, H0], f32, kind="ExternalInput")
    e1s = nc.dram_tensor("e1s", [RS_ROWS, H1], f32, kind="ExternalInput")
    e2s = nc.dram_tensor("e2s", [RS_ROWS, D_OUT_PAD], f32, kind="ExternalInput")
    hb0s = nc.dram_tensor("hb0s", [RS_ROWS, H0], f32, kind="ExternalInput")
    hb1s = nc.dram_tensor("hb1s", [RS_ROWS, H1], f32, kind="ExternalInput")
    hb2s = nc.dram_tensor("hb2s", [RS_ROWS, D_OUT_PAD], f32, kind="ExternalInput")

    y_out = nc.dram_tensor("y_out", [BL, 2048], f32r, kind="ExternalOutput")
    en0 = nc.dram_tensor("en0", [RS_ROWS, H0], f32, kind="ExternalOutput")
    en1 = nc.dram_tensor("en1", [RS_ROWS, H1], f32, kind="ExternalOutput")
    en2 = nc.dram_tensor("en2", [RS_ROWS, D_OUT_PAD], f32, kind="ExternalOutput")
    hbn0 = nc.dram_tensor("hbn0", [RS_ROWS, H0], f32, kind="ExternalOutput")
    hbn1 = nc.dram_tensor("hbn1", [RS_ROWS, H1], f32, kind="ExternalOutput")
    hbn2 = nc.dram_tensor("hbn2", [RS_ROWS, D_OUT_PAD], f32, kind="ExternalOutput")
    wn_out = nc.dram_tensor("wn_out", [1, 1], f32, kind="ExternalOutput")
    dopa_out = nc.dram_tensor("dopa_out", [1, 1], f32, kind="ExternalOutput")

    groups = [list(range(N_CORES))]

    with tile.TileContext(nc) as tc:
        with ExitStack() as octx:
            dram = octx.enter_context(tc.tile_pool(name="dram", bufs=1, space="DRAM"))
            relu_pool = octx.enter_context(tc.tile_pool(name="relu_pool", bufs=3))
            _CACHE["relu_pool"] = relu_pool

            h0x = dram.tile([BL, H0], f32r)    # pre-activation, batch-major
            h0b = dram.tile([BL, H0], f32r)    # relu, batch-major
            h0T = dram.tile([H0, BL], f32r)    # relu, feature-major
            h1x = dram.tile([BL, H1], f32r)
            h1b = dram.tile([BL, H1], f32r)
            h1T = dram.tile([H1, BL], f32r)
            y1 = dram.tile([BL, D_OUT_PAD], f32r)
            p0 = dram.tile([D_IN, H0], f32)
            p1 = dram.tile([H0, H1], f32)
            p2 = dram.tile([H1, D_OUT_PAD], f32)
            rs0 = dram.tile([RS_ROWS, H0], f32)
            rs1 = dram.tile([RS_ROWS, H1], f32)
            rs2 = dram.tile([RS_ROWS, D_OUT_PAD], f32)
            ar_in = dram.tile([1, 2], f32)
            ar_out = dram.tile([1, 2], f32, addr_space="Shared")

            idsb_pool = octx.enter_context(tc.tile_pool(name="idsb_pool", bufs=1))
            idsb = idsb_pool.tile([128, 128], f32r)
            nc.sync.dma_start(idsb[:], ident.ap())

            # shared operand pools across all GEMM phases: lets Tile prefetch
            # phase N+1's tiles while phase N computes instead of serializing
            # on per-phase pool release
            gemm_stack = ExitStack()
            kxm_pool = gemm_stack.enter_context(tc.tile_pool(name="kxm_pool", bufs=6))
            kxn_pool = gemm_stack.enter_context(tc.tile_pool(name="kxn_pool", bufs=6))
            mmkw = dict(kxm_pool=kxm_pool, kxn_pool=kxn_pool)

            # ---- L1: h0x = x @ w0, plus relu copy h0b
            matmul_tile_kernel(
                tc, xT.ap(), w0.ap(), h0x,
                post_mxn_tile_fn=_store_relu_post(h0b),
                **mmkw,
            )

            # ---- transpose: h0T = relu(h0x).T
            with ExitStack() as ctx:
                _transpose_relu(nc, tc, ctx, h0x, h0T, idsb, BL, H0, "t0")

            # ---- trace 0 partial: p0 = x.T @ h0x
            matmul_tile_kernel(tc, x.ap(), h0x, p0, **mmkw)
            nc.gpsimd.collective_compute(
                "ReduceScatter",
                mybir.AluOpType.add,
                replica_groups=groups,
                ins=[p0.opt()],
                outs=[rs0.opt()],
            )

            # ---- L2: h1x = relu(h0x) @ w1 = h0T.T @ w1
            matmul_tile_kernel(
                tc, h0T, w1.ap(), h1x,
                post_mxn_tile_fn=_store_relu_post(h1b),
                **mmkw,
            )

            # ---- transpose: h1T = relu(h1x).T
            with ExitStack() as ctx:
                _transpose_relu(nc, tc, ctx, h1x, h1T, idsb, BL, H1, "t1")

            # ---- trace 1 partial: p1 = relu(h0x).T @ h1x
            matmul_tile_kernel(tc, h0b, h1x, p1, **mmkw)
            nc.gpsimd.collective_compute(
                "ReduceScatter",
                mybir.AluOpType.add,
                replica_groups=groups,
                ins=[p1.opt()],
                outs=[rs1.opt()],
            )

            # ---- L3: y1 = relu(h1x) @ w2, with the y output columns written
            # straight from the product tiles (saves a DRAM round-trip copy)
            def _store_y(nc_, sbuf, md, _):
                n0 = md.n_tile_idx * md.n_tile
                if n0 >= 2048:
                    return
                ns = min(md.n_slice_size, 2048 - n0)
                m0 = md.m_tile_idx * md.m_tile
                dst = y_out.ap()[ds(m0, md.m_tile), ds(n0, ns)].rearrange(
                    "(s p) n -> p s n", p=128
                )
                nc_.sync.dma_start(dst, sbuf[:, :, :ns])

            matmul_tile_kernel(
                tc, h1T, w2.ap(), y1,
                post_mxn_tile_fn=_store_y,
                **mmkw,
            )

            # ---- trace 2 partial: p2 = relu(h1x).T @ y1
            matmul_tile_kernel(tc, h1b, y1, p2, **mmkw)
            gemm_stack.close()
            nc.gpsimd.collective_compute(
                "ReduceScatter",
                mybir.AluOpType.add,
                replica_groups=groups,
                ins=[p2.opt()],
                outs=[rs2.opt()],
            )

            # ---- local tanh sums for dopa (y1 col 2047) / W_new (col 2048)
            with ExitStack() as ctx:
                sc_pool = ctx.enter_context(tc.tile_pool(name="sc_pool", bufs=1))
                sc_psum = ctx.enter_context(
                    tc.tile_pool(name="sc_psum", bufs=1, space="PSUM")
                )
                ycols = sc_pool.tile([128, BL // 128, 2], f32r)
                nc.sync.dma_start(
                    ycols[:],
                    y1[:, 2047:2049].rearrange("(a p) c -> p a c", p=128),
                )
                # f32 (not f32r) tiles: this matmul runs in plain fp32 so the
                # engine-written tanh output is a legal operand
                tcols = sc_pool.tile([128, BL // 128, 2], f32)
                nc.scalar.activation(
                    tcols[:], ycols[:], mybir.ActivationFunctionType.Tanh
                )
                ones = sc_pool.tile([128, 1], f32)
                nc.any.memset(ones[:], 1.0)
                psc = sc_psum.tile([1, 2], f32)
                for a in range(BL // 128):
                    nc.tensor.matmul(
                        psc[:],
                        ones[:],
                        tcols[:, a, :],
                        start=(a == 0),
                        stop=(a == BL // 128 - 1),
                    )
                ssum = sc_pool.tile([1, 2], f32)
                nc.any.tensor_copy(ssum[:], psc[:])
                nc.sync.dma_start(ar_in[:], ssum[:])

            nc.gpsimd.collective_compute(
                "AllReduce",
                mybir.AluOpType.add,
                replica_groups=groups,
                ins=[ar_in.opt()],
                outs=[ar_out.opt()],
            )

            # ---- scalars: divide by B, write outputs, broadcast dopa
            with ExitStack() as ctx:
                sc2_pool = ctx.enter_context(tc.tile_pool(name="sc2_pool", bufs=1))
                gsum = sc2_pool.tile([1, 2], f32)
                nc.sync.dma_start(gsum[:], ar_out[:])
                gmean = sc2_pool.tile([1, 2], f32)
                nc.scalar.mul(gmean[:], gsum[:], 1.0 / B)
                nc.sync.dma_start(dopa_out.ap(), gmean[:, 0:1])
                nc.sync.dma_start(wn_out.ap(), gmean[:, 1:2])
                # round-trip through DRAM to broadcast dopa across partitions
                dopa_dram = dram.tile([1, 1], f32)
                nc.sync.dma_start(dopa_dram[:], gmean[:, 0:1])
                dbr = sc2_pool.tile([128, 1], f32)
                nc.sync.dma_start(dbr[:], dopa_dram[:].partition_broadcast(128))

                # ---- epilogue: e_n = (1-a)*e + a*P ; heb_n = clip(heb + dopa*e_n)
                ep_pool = ctx.enter_context(tc.tile_pool(name="ep_pool", bufs=3))
                for rs_t, e_t, hb_t, en_t, hbn_t, C in (
                    (rs0, e0s, hb0s, en0, hbn0, H0),
                    (rs1, e1s, hb1s, en1, hbn1, H1),
                    (rs2, e2s, hb2s, en2, hbn2, D_OUT_PAD),
                ):
                    for r in range(RS_ROWS // 128):
                        rsl = slice(r * 128, (r + 1) * 128)
                        pt = ep_pool.tile([128, C], f32, tag="pt")
                        et = ep_pool.tile([128, C], f32, tag="et")
                        hbt = ep_pool.tile([128, C], f32, tag="hbt")
                        nc.sync.dma_start(pt[:], rs_t[rsl, :])
                        nc.sync.dma_start(et[:], e_t.ap()[rsl, :])
                        nc.sync.dma_start(hbt[:], hb_t.ap()[rsl, :])
                        ent = ep_pool.tile([128, C], f32, tag="ent")
                        nc.vector.tensor_scalar_mul(ent[:], pt[:], ALPHA)
                        nc.vector.tensor_scalar_mul(et[:], et[:], 1.0 - ALPHA)
                        nc.vector.tensor_tensor(
                            ent[:], ent[:], et[:], mybir.AluOpType.add
                        )
                        nc.sync.dma_start(en_t.ap()[rsl, :], ent[:])
                        hdt = ep_pool.tile([128, C], f32, tag="hdt")
                        nc.vector.tensor_scalar_mul(hdt[:], ent[:], dbr[:, 0:1])
                        nc.vector.tensor_tensor(
                            hdt[:], hdt[:], hbt[:], mybir.AluOpType.add
                        )
                        nc.vector.tensor_scalar(
                            hdt[:],
                            hdt[:],
                            CLAMP,
                            -CLAMP,
                            op0=mybir.AluOpType.min,
                            op1=mybir.AluOpType.max,
                        )
                        nc.sync.dma_start(hbn_t.ap()[rsl, :], hdt[:])

    nc.compile()
    _CACHE["nc"] = nc
    return nc


def _prep_in_maps(x, x2h0, h02h1, h12y, e_x2h0, e_h02h1, e_h12y,
                  heb_x2h0, heb_h02h1, heb_h12y, W):
    W_ = float(np.asarray(W))
    f = np.float32

    def eff(w, heb):
        w = np.asarray(w, dtype=f)
        if W_ != 0.0:
            w = (w + f(W_) * np.asarray(heb, dtype=f)).astype(f)
        return np.ascontiguousarray(w)

    def padcols(a, width):
        a = np.asarray(a, dtype=f)
        out = np.zeros((a.shape[0], width), dtype=f)
        out[:, : a.shape[1]] = a
        return out

    w0 = eff(x2h0, heb_x2h0)
    w1 = eff(h02h1, heb_h02h1)
    w2 = padcols(eff(h12y, heb_h12y), D_OUT_PAD)

    x = np.asarray(x, dtype=f)
    e0 = np.asarray(e_x2h0, dtype=f)
    e1 = np.asarray(e_h02h1, dtype=f)
    e2 = padcols(e_h12y, D_OUT_PAD)
    hb0 = np.asarray(heb_x2h0, dtype=f)
    hb1 = np.asarray(heb_h02h1, dtype=f)
    hb2 = padcols(heb_h12y, D_OUT_PAD)
    ident = np.eye(128, dtype=f)

    in_maps = []
    for c in range(N_CORES):
        bsl = slice(c * BL, (c + 1) * BL)
        rsl = slice(c * RS_ROWS, (c + 1) * RS_ROWS)
        xc = np.ascontiguousarray(x[bsl])
        in_maps.append({
            "xT": np.ascontiguousarray(xc.T),
            "x": xc,
            "w0": w0,
            "w1": w1,
            "w2": w2,
            "ident": ident,
            "e0s": np.ascontiguousarray(e0[rsl]),
            "e1s": np.ascontiguousarray(e1[rsl]),
            "e2s": np.ascontiguousarray(e2[rsl]),
            "hb0s": np.ascontiguousarray(hb0[rsl]),
            "hb1s": np.ascontiguousarray(hb1[rsl]),
            "hb2s": np.ascontiguousarray(hb2[rsl]),
        })
    return in_maps


def _assemble(results):
    cat = lambda key: np.concatenate(
        [results[c][key] for c in range(N_CORES)], axis=0
    )
    y = cat("y_out")
    e0 = cat("en0")
    e1 = cat("en1")
    e2 = cat("en2")[:, :D_OUT]
    h0 = cat("hbn0")
    h1 = cat("hbn1")
    h2 = cat("hbn2")[:, :D_OUT]
    w_new = np.float32(results[0]["wn_out"][0, 0])
    dopa = np.float32(results[0]["dopa_out"][0, 0])
    return (y, w_new, dopa,
            np.ascontiguousarray(e0), np.ascontiguousarray(e1),
            np.ascontiguousarray(e2), np.ascontiguousarray(h0),
            np.ascontiguousarray(h1), np.ascontiguousarray(h2))


def run(inputs, trace=False, **kw):
    nc = build()
    in_maps = _prep_in_maps(**inputs)
    res = bass_utils.run_bass_kernel_spmd(
        nc, in_maps, core_ids=list(range(N_CORES)), trace=trace, **kw
    )
    return _assemble(res.results), res


def kernel(**inputs):
    out, _ = run(inputs)
    return out
